# revision 10
# baseline (speedup 1.0000x reference)
"""Trainium2 Bass kernel for BinaryMLP.

reference:
    h = relu(x @ sign(W1).T + b1)   # [B, 128], x: [B, 196]
    h = relu(h @ sign(W2).T + b2)   # [B, 128]
    h = relu(h @ sign(W3).T + b3)   # [B, 128]
    y = h @ W4.T + b4               # [B, 10]

Strategy (pure data parallel over 8 cores, 65536 rows each):
  - Host: transpose + bf16-cast the x shard -> xT [196, B_core] so the
    contraction dim lands on SBUF partitions and every DMA is contiguous.
    sign(W) is exact in bf16. b4 is added on host.
  - Device: 512-column batch tiles in weight-paired twos. L1 (K=196)
    needs 2 matmuls per tile (128+68 contraction split); L2/L3 are one
    each; consecutive same-weight matmuls skip LDWEIGHTS.
  - x is loaded in [*, 2048] half-pack chunks, emitted on the Sync
    (HWDGE) queue in strict consumption order with 8-deep buffering so
    the reuse guard of the FIFO-head DMA is always satisfied at entry
    (the v1 kernel's stalls came from prefetch DMAs blocking the Sync
    FIFO head on tight buffer-reuse waits).
  - L2/L3 matmul pairs write the two banks of a [128,1024] PSUM tile;
    ONE ScalarE/VectorE op evacuates both banks, amortizing the ~200ns
    per-op fixed cost. PSUM: 3 (L1) + 2 (L2) + 2 (L3) + 1 (head).
  - Evac engines alternate so ScalarE/VectorE each carry ~half.
  - Head (M=10): packed 8 tiles per PSUM bank using 4x column tiling
    (tile_position=(0,32u)) x 2 accumulated zero-masked W4 variants.
    Head waves are delayed 2 steps behind L3 so they never wait on a
    same-step evacuation. Output strip layout yTS[128, .] in bf16; the
    host unscrambles and adds b4 in fp32.
"""

import numpy as np
import ml_dtypes

import concourse.bass as bass
from concourse.bass import _add_dep_helper
import concourse.mybir as mybir
import concourse.tile as tile
from concourse import bacc
from concourse.bass_utils import run_bass_kernel_spmd

BF16 = ml_dtypes.bfloat16

B_FULL, D_IN, H, D_OUT = 524288, 196, 128, 10
N_CORES = 8
TB = 512          # batch tile = matmul free dim (one PSUM bank of fp32)
PACK = 8          # tiles per head pack / store group
HALF = 4          # tiles per x-load chunk (2048 cols)
K1A = 128
K1B = 128  # second K-split padded 68 -> 128 with zero rows: a [68, N] DMA
           # concentrates its descriptors on 4 of 16 SDMA engines (the
           # engine map is partition-keyed), saturating them at ~25GB/s;
           # a [128, N] transfer spreads evenly across all 16.


def build_nc(b_core: int, n_cores: int = N_CORES, noload_opt: bool = True):
    """Build the per-core Bass program (SPMD: same program on all cores)."""
    dt = mybir.dt
    nc = bacc.Bacc(
        "TRN2", target_bir_lowering=False, debug=False, num_devices=n_cores
    )

    n_tiles = b_core // TB
    assert b_core % (PACK * TB) == 0
    n_packs = n_tiles // PACK
    n_pairs = n_tiles // 2
    n_halves = n_tiles // HALF
    HW = HALF * TB  # 2048 cols per load chunk

    xTa = nc.dram_tensor("xTa", [K1A, b_core], dt.bfloat16, kind="ExternalInput").ap()
    xTb = nc.dram_tensor("xTb", [K1B, b_core], dt.bfloat16, kind="ExternalInput").ap()
    w1t = nc.dram_tensor("w1t", [K1A + K1B, H], dt.bfloat16, kind="ExternalInput").ap()
    w2t = nc.dram_tensor("w2t", [H, H], dt.bfloat16, kind="ExternalInput").ap()
    w3t = nc.dram_tensor("w3t", [H, H], dt.bfloat16, kind="ExternalInput").ap()
    w4a = nc.dram_tensor("w4a", [H, 32], dt.bfloat16, kind="ExternalInput").ap()
    w4b = nc.dram_tensor("w4b", [H, 32], dt.bfloat16, kind="ExternalInput").ap()
    b1d = nc.dram_tensor("b1", [H, 1], dt.float32, kind="ExternalInput").ap()
    b2d = nc.dram_tensor("b2", [H, 1], dt.float32, kind="ExternalInput").ap()
    b3d = nc.dram_tensor("b3", [H, 1], dt.float32, kind="ExternalInput").ap()
    # strip-layout output: row 32u+10j+p, cols pk*TB+c  <->  y[(pk*8+4j+u)*TB+c, p]
    yTS = nc.dram_tensor(
        "yTS", [H, n_packs * TB], dt.bfloat16, kind="ExternalOutput"
    ).ap()

    relu = mybir.ActivationFunctionType.Relu

    with tile.TileContext(nc) as tc:
        with (
            tc.tile_pool(name="wpool", bufs=1) as wpool,
            tc.tile_pool(name="xa", bufs=8) as xa_pool,
            tc.tile_pool(name="xb", bufs=8) as xb_pool,
            tc.tile_pool(name="h1p", bufs=8) as h1_pool,
            tc.tile_pool(name="h2p", bufs=4) as h2_pool,
            tc.tile_pool(name="h3p", bufs=6) as h3_pool,
            tc.tile_pool(name="yo", bufs=3) as y_pool,
            tc.tile_pool(name="ps1", bufs=3, space="PSUM") as ps1,
            tc.tile_pool(name="ps2", bufs=1, space="PSUM") as ps2,
            tc.tile_pool(name="ps3", bufs=1, space="PSUM") as ps3,
            tc.tile_pool(name="ps4", bufs=1, space="PSUM") as ps4,
        ):
            # --- load L1 weights first on the fast HWDGE path (first-MM
            # gate), everything else on gpsimd SWDGE ---
            w1a_sb = wpool.tile([K1A, H], dt.bfloat16)
            nc.sync.dma_start(w1a_sb[:], w1t[0:K1A, :])
            w1b_sb = wpool.tile([K1B, H], dt.bfloat16)
            nc.sync.dma_start(w1b_sb[:], w1t[K1A : K1A + K1B, :])
            w2_sb = wpool.tile([H, H], dt.bfloat16)
            nc.gpsimd.dma_start(w2_sb[:], w2t[:, :])
            w3_sb = wpool.tile([H, H], dt.bfloat16)
            nc.gpsimd.dma_start(w3_sb[:], w3t[:, :])
            w4_sb = [
                wpool.tile([H, 32], dt.bfloat16, tag=f"w4_{j}", name=f"w4_{j}")
                for j in range(2)
            ]
            nc.gpsimd.dma_start(w4_sb[0][:], w4a[:, :])
            nc.gpsimd.dma_start(w4_sb[1][:], w4b[:, :])
            b_sb = []
            for j, bd in enumerate((b1d, b2d, b3d)):
                b = wpool.tile([H, 1], dt.float32, tag=f"b_{j}", name=f"b_{j}")
                nc.gpsimd.dma_start(b[:], bd[:, :])
                b_sb.append(b)

            def relu_evac(use_act, h_out, psum_in, bias_sb):
                if use_act:
                    return nc.scalar.activation(h_out[:], psum_in[:], relu, bias=bias_sb[:])
                else:
                    return nc.vector.tensor_scalar(
                        h_out[:],
                        psum_in[:],
                        bias_sb[:],
                        0.0,
                        mybir.AluOpType.add,
                        mybir.AluOpType.max,
                    )

            noload = []  # matmuls that reuse already-loaded weights
            xa_t: dict = {}
            xb_t: dict = {}
            h1_t: dict = {}
            h2_t: dict = {}
            h3_t: dict = {}

            def emit_load(hh, split=False):
                ch = slice(hh * HW, (hh + 1) * HW)
                xa = xa_pool.tile([K1A, HW], dt.bfloat16, tag="xa", name=f"xa_{hh}")
                xb = xb_pool.tile([K1B, HW], dt.bfloat16, tag="xb", name=f"xb_{hh}")
                if split:
                    # quarter-granularity writes so the first matmuls gate
                    # on a 256KB DMA, not the full half
                    qn = HW // 2
                    for qf in range(2):
                        cq = slice(hh * HW + qf * qn, hh * HW + (qf + 1) * qn)
                        nc.sync.dma_start(xa[:, qf * qn : (qf + 1) * qn], xTa[:, cq])
                        nc.sync.dma_start(xb[:, qf * qn : (qf + 1) * qn], xTb[:, cq])
                else:
                    nc.sync.dma_start(xa[:], xTa[:, ch])
                    nc.sync.dma_start(xb[:], xTb[:, ch])
                xa_t[hh], xb_t[hh] = xa, xb

            last_pe = [None]  # last PE instruction emitted this step

            def stage_A1(i):  # L1 part a for pair i: W1a(t0),W1a(t1)
                hh, sl = divmod(i, 2)
                xa = xa_t[hh]
                base = sl * (2 * TB)
                ps = []
                for q in range(2):
                    t = 2 * i + q
                    o = base + q * TB
                    p1 = ps1.tile([H, TB], dt.float32, tag="p1", name=f"p1_{t}")
                    mm = nc.tensor.matmul(
                        p1[:], w1a_sb[:], xa[:, o : o + TB], start=True, stop=False
                    )
                    if q == 1 and noload_opt:
                        mm.ins.ldweights = False
                        noload.append(mm.ins)
                    last_pe[0] = mm
                    ps.append((t, o, p1))
                return ps

            def stage_A2(i, ps):  # L1 part b: W1b(t0),W1b(t1)
                hh, sl = divmod(i, 2)
                xb = xb_t[hh]
                for qq, (t, o, p1) in enumerate(ps):
                    mm = nc.tensor.matmul(
                        p1[:], w1b_sb[:], xb[:, o : o + TB], start=False, stop=True
                    )
                    if qq == 1 and noload_opt:
                        mm.ins.ldweights = False
                        noload.append(mm.ins)
                    last_pe[0] = mm

            def evacs_A(i, ps):
                for t, o, p1 in ps:
                    h1 = h1_pool.tile([H, TB], dt.bfloat16, tag="h1", name=f"h1_{t}")
                    relu_evac((t + i) % 2 == 0, h1, p1, b_sb[0])
                    h1_t[t] = h1

            def stage_B(i):  # L2 for pair i -> one 2-bank evac
                p2 = ps2.tile([H, 2 * TB], dt.float32, tag="p2", name=f"p2_{i}")
                for q in range(2):
                    t = 2 * i + q
                    h1 = h1_t.pop(t)
                    mm = nc.tensor.matmul(
                        p2[:, q * TB : (q + 1) * TB], w2_sb[:], h1[:],
                        start=True, stop=True,
                    )
                    if q == 1 and noload_opt:
                        mm.ins.ldweights = False
                        noload.append(mm.ins)
                    last_pe[0] = mm
                h2 = h2_pool.tile([H, 2 * TB], dt.bfloat16, tag="h2", name=f"h2_{i}")
                relu_evac(i % 2 == 0, h2, p2, b_sb[1])
                h2_t[i] = h2

            def stage_C(i):  # L3 for pair i -> one 2-bank evac
                p3 = ps3.tile([H, 2 * TB], dt.float32, tag="p3", name=f"p3_{i}")
                h2 = h2_t.pop(i)
                for q in range(2):
                    mm = nc.tensor.matmul(
                        p3[:, q * TB : (q + 1) * TB], w3_sb[:],
                        h2[:, q * TB : (q + 1) * TB],
                        start=True, stop=True,
                    )
                    if q == 1 and noload_opt:
                        mm.ins.ldweights = False
                        noload.append(mm.ins)
                    last_pe[0] = mm
                h3 = h3_pool.tile([H, 2 * TB], dt.bfloat16, tag="h3", name=f"h3_{i}")
                e3 = relu_evac(i % 2 == 1, h3, p3, b_sb[2])
                h3_t[i] = (h3, e3)

            p4_t: dict = {}

            def stage_Hj(pk, j):
                # head burst: variant j covers tiles 4j+u (u=0..3) of the
                # pack = pairs (4pk+2j, 4pk+2j+1), whose L3 evacs are >=2
                # steps old.  The burst is pinned contiguous behind the
                # step's last layer matmul with same-engine ordering deps,
                # so the scheduler cannot scatter the col-group LDWEIGHTS
                # between layer matmuls (each scatter serializes ~107ns).
                if j == 0:
                    p4_t[pk] = ps4.tile([H, TB], dt.float32, tag="p4", name=f"p4_{pk}")
                p4 = p4_t[pk]
                pairs = [4 * pk + 2 * j, 4 * pk + 2 * j + 1]
                hs = []
                e3s = []
                for pr in pairs:
                    h3, e3 = h3_t[pr]
                    hs.append(h3[:, 0:TB])
                    hs.append(h3[:, TB : 2 * TB])
                    e3s.append(e3)
                ldws = []
                for u in range(4):
                    ldw = nc.tensor.ldweights(
                        w4_sb[j][:], tile_position=(0, 32 * u)
                    )
                    for e3 in e3s:
                        _add_dep_helper(ldw.ins, e3.ins, True, "head ldw after e3")
                    if u == 0:
                        if last_pe[0] is not None:
                            _add_dep_helper(
                                ldw.ins, last_pe[0].ins, False, "pin head burst"
                            )
                    else:
                        _add_dep_helper(ldw.ins, ldws[-1].ins, False, "chain ldw")
                    ldws.append(ldw)
                mms = []
                for u in range(4):
                    mm = nc.tensor.matmul(
                        p4[32 * u : 32 * u + 32, :],
                        w4_sb[j][:],
                        hs[u],
                        start=(j == 0),
                        stop=(j == 1),
                        tile_position=(0, 32 * u),
                        skip_group_check=True,
                    )
                    mm.ins.ldweights = False
                    _add_dep_helper(mm.ins, ldws[u].ins, False, "head mm after ldw")
                    prev = mms[-1] if mms else ldws[-1]
                    _add_dep_helper(mm.ins, prev.ins, False, "chain head mm")
                    mms.append(mm)
                last_pe[0] = mms[-1]
                if j == 1:
                    for pr in [4 * pk, 4 * pk + 1] + pairs:
                        h3_t.pop(pr, None)

            def copy_store(pk):
                # one step after the pack's last head wave: the copy's
                # input is already complete, so it cannot block the Scalar
                # FIFO head and delay the critical evacuations behind it.
                p4 = p4_t.pop(pk)
                ysb = y_pool.tile([H, TB], dt.bfloat16, tag="ysb", name=f"ysb_{pk}")
                nc.scalar.copy(ysb[:], p4[:])
                # per-pack stores on GpSimd (SWDGE): small bursts that
                # never block load triggers on the Sync sequencer
                nc.gpsimd.dma_start(yTS[:, pk * TB : (pk + 1) * TB], ysb[:])

            # --- software-pipelined emission ---
            # PE stage order within a step is A, C, B (+ pinned head burst)
            # so the single-buffered ps2/ps3 evacuations get a full step of
            # slack before the next pair's matmuls need the banks back.
            # L1 evacuations (2 steps of slack) are emitted last so the
            # tight L2/L3 evacuations sit ahead of them in engine queues.
            # x halves: prime 6, then 1 every 2 steps, strictly in order.
            PRIME = 6
            emit_load(0, split=True)
            for hh in range(1, min(PRIME, n_halves)):
                emit_load(hh)
            for step in range(n_pairs + 7):
                if step % 2 == 0:
                    hh = PRIME + step // 2
                    if hh < n_halves:
                        emit_load(hh)
                if step < n_pairs:
                    ps_a = stage_A1(step)
                    stage_A2(step, ps_a)
                else:
                    ps_a = None
                ic = step - 4
                if 0 <= ic < n_pairs:
                    stage_C(ic)
                ib = step - 2
                if 0 <= ib < n_pairs:
                    stage_B(ib)
                iq = step - 6
                if 0 <= iq < n_pairs and iq % 2 == 1:
                    stage_Hj(iq // 4, (iq % 4) // 2)
                if ps_a is not None:
                    evacs_A(step, ps_a)
                if iq == n_pairs - 1:
                    # final pack immediately: no critical evacs remain, and
                    # the one-step delay would add ~2.3us to the drain tail
                    copy_store(iq // 4)
                elif iq >= 4 and iq % 4 == 0 and iq // 4 - 1 < n_packs - 1:
                    copy_store(iq // 4 - 1)

    nc.compile()
    if noload_opt:
        try:
            _verify_noload_safety(nc, noload)
        except AssertionError:
            # schedule changed in a way that makes weight reuse unsafe;
            # rebuild without the optimization (correctness first)
            return build_nc(b_core, n_cores, noload_opt=False)
    return nc


def _weights_key(inst, idx):
    ap = inst.ins[idx]
    s = str(ap)
    return s


def _verify_noload_safety(nc, noload):
    """The schedule is static: verify no other weight load lands between a
    ldweights=False matmul and the instruction that loaded its weights."""
    import concourse.mybir as mybir

    noload_ids = {id(i) for i in noload}
    cur = None  # weights key currently in the PE array (full-array loads)
    checked = 0
    insts = []
    for blk in nc.m.functions[0].blocks:
        insts.extend(blk.instructions)
    for inst in insts:
        if inst.engine != mybir.EngineType.PE:
            continue
        kind = type(inst).__name__
        if kind == "InstLdweights":
            tp = getattr(inst, "tile_position", None)
            if not tp or tuple(tp) == (0, 0):
                cur = _weights_key(inst, 0)
            else:
                cur = ("coltile", None)  # partial col-group load
        elif kind == "InstMatmult":
            if id(inst) in noload_ids:
                want = _weights_key(inst, 1)
                assert cur == want, (
                    f"noload matmul {inst.name} expects weights {want}, array has {cur}"
                )
                checked += 1
            elif getattr(inst, "ldweights", None) is False:
                pass  # head matmul: guarded by its own explicit ldw deps
            else:
                tp = getattr(inst, "tile_position", None)
                if not tp or tuple(tp) == (0, 0):
                    cur = _weights_key(inst, 1)
                else:
                    cur = ("coltile", None)
    assert checked == len(noload), (checked, len(noload))


def _prep_core_inputs(x_shard: np.ndarray, weights: dict) -> dict:
    xT = x_shard.T.astype(BF16)
    xTa = np.ascontiguousarray(xT[0:K1A])
    xTb = np.zeros((K1B, x_shard.shape[0]), BF16)
    xTb[0 : D_IN - K1A] = xT[K1A:D_IN]
    return {"xTa": xTa, "xTb": xTb, **weights}


def _prep_weights(W1, b1, W2, b2, W3, b3, W4) -> dict:
    w4a = np.zeros((32, H), np.float32)
    w4a[0:D_OUT] = W4
    w4b = np.zeros((32, H), np.float32)
    w4b[D_OUT : 2 * D_OUT] = W4
    w1p = np.zeros((K1A + K1B, H), np.float32)
    w1p[0:D_IN] = np.sign(W1).T
    return {
        "w1t": w1p.astype(BF16),
        "w2t": np.ascontiguousarray(np.sign(W2).T).astype(BF16),
        "w3t": np.ascontiguousarray(np.sign(W3).T).astype(BF16),
        "w4a": np.ascontiguousarray(w4a.T).astype(BF16),
        "w4b": np.ascontiguousarray(w4b.T).astype(BF16),
        "b1": b1.reshape(H, 1).astype(np.float32),
        "b2": b2.reshape(H, 1).astype(np.float32),
        "b3": b3.reshape(H, 1).astype(np.float32),
    }


def _unscramble(yTS: np.ndarray, b_core: int) -> np.ndarray:
    """yTS [128, n_packs*TB] strip layout -> y_core [b_core, 10] (fp32)."""
    n_packs = b_core // (PACK * TB)
    # yTS[32u+10j+p, pk*TB+c] = y[(pk*8+4j+u)*TB + c, p]
    v = yTS.astype(np.float32).reshape(4, 32, n_packs, TB)[:, :20]  # [u, 10j+p, pk, c]
    v = v.reshape(4, 2, 10, n_packs, TB)  # [u, j, p, pk, c]
    # -> y[pk, j, u, c, p]
    y = v.transpose(3, 1, 0, 4, 2).reshape(b_core, D_OUT)
    return y


_NC_CACHE: dict = {}


def run(x, W1, b1, W2, b2, W3, b3, W4, b4, trace=False, trace_kwargs=None):
    """Run the SPMD kernel on 8 cores; returns (y, BassKernelResults)."""
    x = np.asarray(x, dtype=np.float32)
    b_total = x.shape[0]
    assert b_total % N_CORES == 0
    b_core = b_total // N_CORES

    key = b_core
    if key not in _NC_CACHE:
        _NC_CACHE[key] = build_nc(b_core)
    nc = _NC_CACHE[key]

    weights = _prep_weights(
        np.asarray(W1), np.asarray(b1), np.asarray(W2), np.asarray(b2),
        np.asarray(W3), np.asarray(b3), np.asarray(W4),
    )
    in_maps = [
        _prep_core_inputs(x[c * b_core : (c + 1) * b_core], weights)
        for c in range(N_CORES)
    ]
    res = run_bass_kernel_spmd(
        nc,
        in_maps,
        list(range(N_CORES)),
        trace=trace,
        **(trace_kwargs or {}),
    )
    b4f = np.asarray(b4, dtype=np.float32)
    y = np.empty((b_total, D_OUT), dtype=np.float32)
    for c in range(N_CORES):
        y[c * b_core : (c + 1) * b_core] = _unscramble(res.results[c]["yTS"], b_core)
    y += b4f
    return y, res


def kernel(x, W1, b1, W2, b2, W3, b3, W4, b4):
    y, _ = run(x, W1, b1, W2, b2, W3, b3, W4, b4)
    return y


# revision 12
# speedup vs baseline: 1.1834x; 1.1834x over previous
"""Trainium2 Bass kernel for BinaryMLP.

reference:
    h = relu(x @ sign(W1).T + b1)   # [B, 128], x: [B, 196]
    h = relu(h @ sign(W2).T + b2)   # [B, 128]
    h = relu(h @ sign(W3).T + b3)   # [B, 128]
    y = h @ W4.T + b4               # [B, 10]

Strategy (pure data parallel over 8 cores, 65536 rows each):
  - Host: transpose + bf16-cast the x shard -> xT [196, B_core] so the
    contraction dim lands on SBUF partitions and every DMA is contiguous.
    sign(W) is exact in bf16. b4 is added on host.
  - Device: 512-column batch tiles in weight-paired twos. L1 (K=196)
    needs 2 matmuls per tile (128+68 contraction split); L2/L3 are one
    each; consecutive same-weight matmuls skip LDWEIGHTS.
  - x is loaded in [*, 2048] half-pack chunks, emitted on the Sync
    (HWDGE) queue in strict consumption order with 8-deep buffering so
    the reuse guard of the FIFO-head DMA is always satisfied at entry
    (the v1 kernel's stalls came from prefetch DMAs blocking the Sync
    FIFO head on tight buffer-reuse waits).
  - L2/L3 matmul pairs write the two banks of a [128,1024] PSUM tile;
    ONE ScalarE/VectorE op evacuates both banks, amortizing the ~200ns
    per-op fixed cost. PSUM: 3 (L1) + 2 (L2) + 2 (L3) + 1 (head).
  - Evac engines alternate so ScalarE/VectorE each carry ~half.
  - Head (M=10): packed 8 tiles per PSUM bank using 4x column tiling
    (tile_position=(0,32u)) x 2 accumulated zero-masked W4 variants.
    Head waves are delayed 2 steps behind L3 so they never wait on a
    same-step evacuation. Output strip layout yTS[128, .] in bf16; the
    host unscrambles and adds b4 in fp32.
"""

import numpy as np
import ml_dtypes

import concourse.bass as bass
from concourse.bass import _add_dep_helper
import concourse.mybir as mybir
import concourse.tile as tile
from concourse import bacc
from concourse.bass_utils import run_bass_kernel_spmd

BF16 = ml_dtypes.bfloat16

B_FULL, D_IN, H, D_OUT = 524288, 196, 128, 10
N_CORES = 8
TB = 512          # batch tile = matmul free dim (one PSUM bank of fp32)
PACK = 8          # tiles per head pack / store group
HALF = 4          # tiles per x-load chunk (2048 cols)
K1A = 128
K1B = 128  # second K-split padded 68 -> 128 with zero rows: a [68, N] DMA
           # concentrates its descriptors on 4 of 16 SDMA engines (the
           # engine map is partition-keyed), saturating them at ~25GB/s;
           # a [128, N] transfer spreads evenly across all 16.


def build_nc(b_core: int, n_cores: int = N_CORES, noload_opt: bool = True):
    """Build the per-core Bass program (SPMD: same program on all cores)."""
    dt = mybir.dt
    nc = bacc.Bacc(
        "TRN2", target_bir_lowering=False, debug=False, num_devices=n_cores
    )

    n_tiles = b_core // TB
    assert b_core % (PACK * TB) == 0
    n_packs = n_tiles // PACK
    n_pairs = n_tiles // 2
    n_halves = n_tiles // HALF
    HW = HALF * TB  # 2048 cols per load chunk

    xTa = nc.dram_tensor("xTa", [K1A, b_core], dt.bfloat16, kind="ExternalInput").ap()
    xTb = nc.dram_tensor("xTb", [K1B, b_core], dt.bfloat16, kind="ExternalInput").ap()
    w1t = nc.dram_tensor("w1t", [K1A + K1B, H], dt.bfloat16, kind="ExternalInput").ap()
    w2t = nc.dram_tensor("w2t", [H, H], dt.bfloat16, kind="ExternalInput").ap()
    w3t = nc.dram_tensor("w3t", [H, H], dt.bfloat16, kind="ExternalInput").ap()
    w4a = nc.dram_tensor("w4a", [H, 32], dt.bfloat16, kind="ExternalInput").ap()
    w4b = nc.dram_tensor("w4b", [H, 32], dt.bfloat16, kind="ExternalInput").ap()
    b1d = nc.dram_tensor("b1", [H, 1], dt.float32, kind="ExternalInput").ap()
    b2d = nc.dram_tensor("b2", [H, 1], dt.float32, kind="ExternalInput").ap()
    b3d = nc.dram_tensor("b3", [H, 1], dt.float32, kind="ExternalInput").ap()
    # strip-layout output: row 32u+10j+p, cols pk*TB+c  <->  y[(pk*8+4j+u)*TB+c, p]
    yTS = nc.dram_tensor(
        "yTS", [H, n_packs * TB], dt.bfloat16, kind="ExternalOutput"
    ).ap()

    relu = mybir.ActivationFunctionType.Relu

    with tile.TileContext(nc) as tc:
        with (
            tc.tile_pool(name="wpool", bufs=1) as wpool,
            tc.tile_pool(name="xa", bufs=8) as xa_pool,
            tc.tile_pool(name="xb", bufs=8) as xb_pool,
            tc.tile_pool(name="h1p", bufs=8) as h1_pool,
            tc.tile_pool(name="h2p", bufs=4) as h2_pool,
            tc.tile_pool(name="h3p", bufs=6) as h3_pool,
            tc.tile_pool(name="yo", bufs=3) as y_pool,
            tc.tile_pool(name="ps1", bufs=3, space="PSUM") as ps1,
            tc.tile_pool(name="ps2", bufs=1, space="PSUM") as ps2,
            tc.tile_pool(name="ps3", bufs=1, space="PSUM") as ps3,
            tc.tile_pool(name="ps4", bufs=1, space="PSUM") as ps4,
        ):
            # --- load L1 weights first on the fast HWDGE path (first-MM
            # gate), everything else on gpsimd SWDGE ---
            w1a_sb = wpool.tile([K1A, H], dt.bfloat16)
            nc.sync.dma_start(w1a_sb[:], w1t[0:K1A, :])
            w1b_sb = wpool.tile([K1B, H], dt.bfloat16)
            nc.sync.dma_start(w1b_sb[:], w1t[K1A : K1A + K1B, :])
            w2_sb = wpool.tile([H, H], dt.bfloat16)
            nc.gpsimd.dma_start(w2_sb[:], w2t[:, :])
            w3_sb = wpool.tile([H, H], dt.bfloat16)
            nc.gpsimd.dma_start(w3_sb[:], w3t[:, :])
            w4_sb = [
                wpool.tile([H, 32], dt.bfloat16, tag=f"w4_{j}", name=f"w4_{j}")
                for j in range(2)
            ]
            nc.gpsimd.dma_start(w4_sb[0][:], w4a[:, :])
            nc.gpsimd.dma_start(w4_sb[1][:], w4b[:, :])
            b_sb = []
            for j, bd in enumerate((b1d, b2d, b3d)):
                b = wpool.tile([H, 1], dt.float32, tag=f"b_{j}", name=f"b_{j}")
                nc.gpsimd.dma_start(b[:], bd[:, :])
                b_sb.append(b)

            def relu_evac(use_act, h_out, psum_in, bias_sb):
                if use_act:
                    return nc.scalar.activation(h_out[:], psum_in[:], relu, bias=bias_sb[:])
                else:
                    return nc.vector.tensor_scalar(
                        h_out[:],
                        psum_in[:],
                        bias_sb[:],
                        0.0,
                        mybir.AluOpType.add,
                        mybir.AluOpType.max,
                    )

            noload = []  # matmuls that reuse already-loaded weights
            xa_t: dict = {}
            xb_t: dict = {}
            h1_t: dict = {}
            h2_t: dict = {}
            h3_t: dict = {}

            def emit_load(hh, split=False):
                ch = slice(hh * HW, (hh + 1) * HW)
                xa = xa_pool.tile([K1A, HW], dt.bfloat16, tag="xa", name=f"xa_{hh}")
                xb = xb_pool.tile([K1B, HW], dt.bfloat16, tag="xb", name=f"xb_{hh}")
                if split:
                    # quarter-granularity writes so the first matmuls gate
                    # on a 256KB DMA, not the full half
                    qn = HW // 2
                    for qf in range(2):
                        cq = slice(hh * HW + qf * qn, hh * HW + (qf + 1) * qn)
                        nc.sync.dma_start(xa[:, qf * qn : (qf + 1) * qn], xTa[:, cq])
                        nc.sync.dma_start(xb[:, qf * qn : (qf + 1) * qn], xTb[:, cq])
                else:
                    nc.sync.dma_start(xa[:], xTa[:, ch])
                    nc.sync.dma_start(xb[:], xTb[:, ch])
                xa_t[hh], xb_t[hh] = xa, xb

            last_pe = [None]  # last PE instruction emitted this step

            def stage_A1(i):  # L1 part a for pair i: W1a(t0),W1a(t1)
                hh, sl = divmod(i, 2)
                xa = xa_t[hh]
                base = sl * (2 * TB)
                ps = []
                for q in range(2):
                    t = 2 * i + q
                    o = base + q * TB
                    p1 = ps1.tile([H, TB], dt.float32, tag="p1", name=f"p1_{t}")
                    mm = nc.tensor.matmul(
                        p1[:], w1a_sb[:], xa[:, o : o + TB], start=True, stop=False
                    )
                    if q == 1 and noload_opt:
                        mm.ins.ldweights = False
                        noload.append(mm.ins)
                    last_pe[0] = mm
                    ps.append((t, o, p1))
                return ps

            def stage_A2(i, ps):  # L1 part b: W1b(t0),W1b(t1)
                hh, sl = divmod(i, 2)
                xb = xb_t[hh]
                for qq, (t, o, p1) in enumerate(ps):
                    mm = nc.tensor.matmul(
                        p1[:], w1b_sb[:], xb[:, o : o + TB], start=False, stop=True
                    )
                    if qq == 1 and noload_opt:
                        mm.ins.ldweights = False
                        noload.append(mm.ins)
                    last_pe[0] = mm

            def evacs_A(i, ps):
                for t, o, p1 in ps:
                    h1 = h1_pool.tile([H, TB], dt.bfloat16, tag="h1", name=f"h1_{t}")
                    use_act = True if i % 8 == 7 else (t + i) % 2 == 0
                    relu_evac(use_act, h1, p1, b_sb[0])
                    h1_t[t] = h1

            def stage_B(i):  # L2 for pair i -> one 2-bank evac
                p2 = ps2.tile([H, 2 * TB], dt.float32, tag="p2", name=f"p2_{i}")
                for q in range(2):
                    t = 2 * i + q
                    h1 = h1_t.pop(t)
                    mm = nc.tensor.matmul(
                        p2[:, q * TB : (q + 1) * TB], w2_sb[:], h1[:],
                        start=True, stop=True,
                    )
                    if q == 1 and noload_opt:
                        mm.ins.ldweights = False
                        noload.append(mm.ins)
                    last_pe[0] = mm
                h2 = h2_pool.tile([H, 2 * TB], dt.bfloat16, tag="h2", name=f"h2_{i}")
                relu_evac(i % 2 == 0, h2, p2, b_sb[1])
                h2_t[i] = h2

            def stage_C(i):  # L3 for pair i -> one 2-bank evac
                p3 = ps3.tile([H, 2 * TB], dt.float32, tag="p3", name=f"p3_{i}")
                h2 = h2_t.pop(i)
                for q in range(2):
                    mm = nc.tensor.matmul(
                        p3[:, q * TB : (q + 1) * TB], w3_sb[:],
                        h2[:, q * TB : (q + 1) * TB],
                        start=True, stop=True,
                    )
                    if q == 1 and noload_opt:
                        mm.ins.ldweights = False
                        noload.append(mm.ins)
                    last_pe[0] = mm
                h3 = h3_pool.tile([H, 2 * TB], dt.bfloat16, tag="h3", name=f"h3_{i}")
                e3 = relu_evac(i % 2 == 1, h3, p3, b_sb[2])
                h3_t[i] = (h3, e3)

            p4_t: dict = {}

            def stage_Hj(pk, j):
                # head burst: variant j covers tiles 4j+u (u=0..3) of the
                # pack = pairs (4pk+2j, 4pk+2j+1), whose L3 evacs are >=2
                # steps old.  The burst is pinned contiguous behind the
                # step's last layer matmul with same-engine ordering deps,
                # so the scheduler cannot scatter the col-group LDWEIGHTS
                # between layer matmuls (each scatter serializes ~107ns).
                if j == 0:
                    p4_t[pk] = ps4.tile([H, TB], dt.float32, tag="p4", name=f"p4_{pk}")
                p4 = p4_t[pk]
                pairs = [4 * pk + 2 * j, 4 * pk + 2 * j + 1]
                hs = []
                e3s = []
                for pr in pairs:
                    h3, e3 = h3_t[pr]
                    hs.append(h3[:, 0:TB])
                    hs.append(h3[:, TB : 2 * TB])
                    e3s.append(e3)
                ldws = []
                for u in range(4):
                    ldw = nc.tensor.ldweights(
                        w4_sb[j][:], tile_position=(0, 32 * u)
                    )
                    for e3 in e3s:
                        _add_dep_helper(ldw.ins, e3.ins, True, "head ldw after e3")
                    if u == 0:
                        if last_pe[0] is not None:
                            _add_dep_helper(
                                ldw.ins, last_pe[0].ins, False, "pin head burst"
                            )
                    else:
                        _add_dep_helper(ldw.ins, ldws[-1].ins, False, "chain ldw")
                    ldws.append(ldw)
                mms = []
                for u in range(4):
                    mm = nc.tensor.matmul(
                        p4[32 * u : 32 * u + 32, :],
                        w4_sb[j][:],
                        hs[u],
                        start=(j == 0),
                        stop=(j == 1),
                        tile_position=(0, 32 * u),
                        skip_group_check=True,
                    )
                    mm.ins.ldweights = False
                    _add_dep_helper(mm.ins, ldws[u].ins, False, "head mm after ldw")
                    prev = mms[-1] if mms else ldws[-1]
                    _add_dep_helper(mm.ins, prev.ins, False, "chain head mm")
                    mms.append(mm)
                last_pe[0] = mms[-1]
                if j == 1:
                    for pr in [4 * pk, 4 * pk + 1] + pairs:
                        h3_t.pop(pr, None)

            def copy_store(pk):
                # one step after the pack's last head wave: the copy's
                # input is already complete, so it cannot block the Scalar
                # FIFO head and delay the critical evacuations behind it.
                p4 = p4_t.pop(pk)
                ysb = y_pool.tile([H, TB], dt.bfloat16, tag="ysb", name=f"ysb_{pk}")
                nc.scalar.copy(ysb[:], p4[:])
                # per-pack stores on GpSimd (SWDGE): small bursts that
                # never block load triggers on the Sync sequencer
                nc.gpsimd.dma_start(yTS[:, pk * TB : (pk + 1) * TB], ysb[:])

            # --- software-pipelined emission ---
            # PE stage order within a step is A, C, B (+ pinned head burst)
            # so the single-buffered ps2/ps3 evacuations get a full step of
            # slack before the next pair's matmuls need the banks back.
            # L1 evacuations (2 steps of slack) are emitted last so the
            # tight L2/L3 evacuations sit ahead of them in engine queues.
            # x halves: prime 6, then 1 every 2 steps, strictly in order.
            PRIME = 6
            emit_load(0, split=True)
            for hh in range(1, min(PRIME, n_halves)):
                emit_load(hh)
            for step in range(n_pairs + 7):
                if step % 2 == 0:
                    hh = PRIME + step // 2
                    if hh < n_halves:
                        emit_load(hh)
                if step < n_pairs:
                    ps_a = stage_A1(step)
                    stage_A2(step, ps_a)
                else:
                    ps_a = None
                ic = step - 4
                if 0 <= ic < n_pairs:
                    stage_C(ic)
                ib = step - 2
                if 0 <= ib < n_pairs:
                    stage_B(ib)
                iq = step - 6
                if 0 <= iq < n_pairs and iq % 2 == 1:
                    stage_Hj(iq // 4, (iq % 4) // 2)
                if ps_a is not None:
                    evacs_A(step, ps_a)
                if iq == n_pairs - 1:
                    copy_store(iq // 4)  # final pack: nothing left to block
                elif iq >= 4 and iq % 4 == 0 and iq // 4 - 1 < n_packs - 1:
                    copy_store(iq // 4 - 1)

    nc.compile()
    if noload_opt:
        try:
            _verify_noload_safety(nc, noload)
        except AssertionError:
            # schedule changed in a way that makes weight reuse unsafe;
            # rebuild without the optimization (correctness first)
            return build_nc(b_core, n_cores, noload_opt=False)
    return nc


def _weights_key(inst, idx):
    ap = inst.ins[idx]
    s = str(ap)
    return s


def _verify_noload_safety(nc, noload):
    """The schedule is static: verify no other weight load lands between a
    ldweights=False matmul and the instruction that loaded its weights."""
    import concourse.mybir as mybir

    noload_ids = {id(i) for i in noload}
    cur = None  # weights key currently in the PE array (full-array loads)
    checked = 0
    insts = []
    for blk in nc.m.functions[0].blocks:
        insts.extend(blk.instructions)
    for inst in insts:
        if inst.engine != mybir.EngineType.PE:
            continue
        kind = type(inst).__name__
        if kind == "InstLdweights":
            tp = getattr(inst, "tile_position", None)
            if not tp or tuple(tp) == (0, 0):
                cur = _weights_key(inst, 0)
            else:
                cur = ("coltile", None)  # partial col-group load
        elif kind == "InstMatmult":
            if id(inst) in noload_ids:
                want = _weights_key(inst, 1)
                assert cur == want, (
                    f"noload matmul {inst.name} expects weights {want}, array has {cur}"
                )
                checked += 1
            elif getattr(inst, "ldweights", None) is False:
                pass  # head matmul: guarded by its own explicit ldw deps
            else:
                tp = getattr(inst, "tile_position", None)
                if not tp or tuple(tp) == (0, 0):
                    cur = _weights_key(inst, 1)
                else:
                    cur = ("coltile", None)
    assert checked == len(noload), (checked, len(noload))


def _prep_core_inputs(x_shard: np.ndarray, weights: dict) -> dict:
    xT = x_shard.T.astype(BF16)
    xTa = np.ascontiguousarray(xT[0:K1A])
    xTb = np.zeros((K1B, x_shard.shape[0]), BF16)
    xTb[0 : D_IN - K1A] = xT[K1A:D_IN]
    return {"xTa": xTa, "xTb": xTb, **weights}


def _prep_weights(W1, b1, W2, b2, W3, b3, W4) -> dict:
    w4a = np.zeros((32, H), np.float32)
    w4a[0:D_OUT] = W4
    w4b = np.zeros((32, H), np.float32)
    w4b[D_OUT : 2 * D_OUT] = W4
    w1p = np.zeros((K1A + K1B, H), np.float32)
    w1p[0:D_IN] = np.sign(W1).T
    return {
        "w1t": w1p.astype(BF16),
        "w2t": np.ascontiguousarray(np.sign(W2).T).astype(BF16),
        "w3t": np.ascontiguousarray(np.sign(W3).T).astype(BF16),
        "w4a": np.ascontiguousarray(w4a.T).astype(BF16),
        "w4b": np.ascontiguousarray(w4b.T).astype(BF16),
        "b1": b1.reshape(H, 1).astype(np.float32),
        "b2": b2.reshape(H, 1).astype(np.float32),
        "b3": b3.reshape(H, 1).astype(np.float32),
    }


def _unscramble(yTS: np.ndarray, b_core: int) -> np.ndarray:
    """yTS [128, n_packs*TB] strip layout -> y_core [b_core, 10] (fp32)."""
    n_packs = b_core // (PACK * TB)
    # yTS[32u+10j+p, pk*TB+c] = y[(pk*8+4j+u)*TB + c, p]
    v = yTS.astype(np.float32).reshape(4, 32, n_packs, TB)[:, :20]  # [u, 10j+p, pk, c]
    v = v.reshape(4, 2, 10, n_packs, TB)  # [u, j, p, pk, c]
    # -> y[pk, j, u, c, p]
    y = v.transpose(3, 1, 0, 4, 2).reshape(b_core, D_OUT)
    return y


_NC_CACHE: dict = {}


def run(x, W1, b1, W2, b2, W3, b3, W4, b4, trace=False, trace_kwargs=None):
    """Run the SPMD kernel on 8 cores; returns (y, BassKernelResults)."""
    x = np.asarray(x, dtype=np.float32)
    b_total = x.shape[0]
    assert b_total % N_CORES == 0
    b_core = b_total // N_CORES

    key = b_core
    if key not in _NC_CACHE:
        _NC_CACHE[key] = build_nc(b_core)
    nc = _NC_CACHE[key]

    weights = _prep_weights(
        np.asarray(W1), np.asarray(b1), np.asarray(W2), np.asarray(b2),
        np.asarray(W3), np.asarray(b3), np.asarray(W4),
    )
    in_maps = [
        _prep_core_inputs(x[c * b_core : (c + 1) * b_core], weights)
        for c in range(N_CORES)
    ]
    res = run_bass_kernel_spmd(
        nc,
        in_maps,
        list(range(N_CORES)),
        trace=trace,
        **(trace_kwargs or {}),
    )
    b4f = np.asarray(b4, dtype=np.float32)
    y = np.empty((b_total, D_OUT), dtype=np.float32)
    for c in range(N_CORES):
        y[c * b_core : (c + 1) * b_core] = _unscramble(res.results[c]["yTS"], b_core)
    y += b4f
    return y, res


def kernel(x, W1, b1, W2, b2, W3, b3, W4, b4):
    y, _ = run(x, W1, b1, W2, b2, W3, b3, W4, b4)
    return y


# revision 14
# speedup vs baseline: 1.1877x; 1.0036x over previous
"""Trainium2 Bass kernel for BinaryMLP.

reference:
    h = relu(x @ sign(W1).T + b1)   # [B, 128], x: [B, 196]
    h = relu(h @ sign(W2).T + b2)   # [B, 128]
    h = relu(h @ sign(W3).T + b3)   # [B, 128]
    y = h @ W4.T + b4               # [B, 10]

Strategy (pure data parallel over 8 cores, 65536 rows each):
  - Host: transpose + bf16-cast the x shard -> xT [196, B_core] so the
    contraction dim lands on SBUF partitions and every DMA is contiguous.
    sign(W) is exact in bf16. b4 is added on host.
  - Device: 512-column batch tiles in weight-paired twos. L1 (K=196)
    needs 2 matmuls per tile (128+68 contraction split); L2/L3 are one
    each; consecutive same-weight matmuls skip LDWEIGHTS.
  - x is loaded in [*, 2048] half-pack chunks, emitted on the Sync
    (HWDGE) queue in strict consumption order with 8-deep buffering so
    the reuse guard of the FIFO-head DMA is always satisfied at entry
    (the v1 kernel's stalls came from prefetch DMAs blocking the Sync
    FIFO head on tight buffer-reuse waits).
  - L2/L3 matmul pairs write the two banks of a [128,1024] PSUM tile;
    ONE ScalarE/VectorE op evacuates both banks, amortizing the ~200ns
    per-op fixed cost. PSUM: 3 (L1) + 2 (L2) + 2 (L3) + 1 (head).
  - Evac engines alternate so ScalarE/VectorE each carry ~half.
  - Head (M=10): packed 8 tiles per PSUM bank using 4x column tiling
    (tile_position=(0,32u)) x 2 accumulated zero-masked W4 variants.
    Head waves are delayed 2 steps behind L3 so they never wait on a
    same-step evacuation. Output strip layout yTS[128, .] in bf16; the
    host unscrambles and adds b4 in fp32.
"""

import numpy as np
import ml_dtypes

import concourse.bass as bass
from concourse.bass import _add_dep_helper
import concourse.mybir as mybir
import concourse.tile as tile
from concourse import bacc
from concourse.bass_utils import run_bass_kernel_spmd

BF16 = ml_dtypes.bfloat16

B_FULL, D_IN, H, D_OUT = 524288, 196, 128, 10
N_CORES = 8
TB = 512          # batch tile = matmul free dim (one PSUM bank of fp32)
PACK = 8          # tiles per head pack / store group
HALF = 4          # tiles per x-load chunk (2048 cols)
K1A = 128
K1B = 128  # second K-split padded 68 -> 128 with zero rows: a [68, N] DMA
           # concentrates its descriptors on 4 of 16 SDMA engines (the
           # engine map is partition-keyed), saturating them at ~25GB/s;
           # a [128, N] transfer spreads evenly across all 16.


def build_nc(b_core: int, n_cores: int = N_CORES, noload_opt: bool = True):
    """Build the per-core Bass program (SPMD: same program on all cores)."""
    dt = mybir.dt
    nc = bacc.Bacc(
        "TRN2", target_bir_lowering=False, debug=False, num_devices=n_cores
    )

    n_tiles = b_core // TB
    assert b_core % (PACK * TB) == 0
    n_packs = n_tiles // PACK
    n_pairs = n_tiles // 2
    n_halves = n_tiles // HALF
    HW = HALF * TB  # 2048 cols per load chunk

    xTa = nc.dram_tensor("xTa", [K1A, b_core], dt.bfloat16, kind="ExternalInput").ap()
    xTb = nc.dram_tensor("xTb", [K1B, b_core], dt.bfloat16, kind="ExternalInput").ap()
    w1t = nc.dram_tensor("w1t", [K1A + K1B, H], dt.bfloat16, kind="ExternalInput").ap()
    w2t = nc.dram_tensor("w2t", [H, H], dt.bfloat16, kind="ExternalInput").ap()
    w3t = nc.dram_tensor("w3t", [H, H], dt.bfloat16, kind="ExternalInput").ap()
    w4a = nc.dram_tensor("w4a", [H, 32], dt.bfloat16, kind="ExternalInput").ap()
    w4b = nc.dram_tensor("w4b", [H, 32], dt.bfloat16, kind="ExternalInput").ap()
    b1d = nc.dram_tensor("b1", [H, 1], dt.float32, kind="ExternalInput").ap()
    b2d = nc.dram_tensor("b2", [H, 1], dt.float32, kind="ExternalInput").ap()
    b3d = nc.dram_tensor("b3", [H, 1], dt.float32, kind="ExternalInput").ap()
    # strip-layout output: row 32u+10j+p, cols pk*TB+c  <->  y[(pk*8+4j+u)*TB+c, p]
    yTS = nc.dram_tensor(
        "yTS", [H, n_packs * TB], dt.bfloat16, kind="ExternalOutput"
    ).ap()

    relu = mybir.ActivationFunctionType.Relu

    with tile.TileContext(nc) as tc:
        with (
            tc.tile_pool(name="wpool", bufs=1) as wpool,
            tc.tile_pool(name="xa", bufs=8) as xa_pool,
            tc.tile_pool(name="xb", bufs=8) as xb_pool,
            tc.tile_pool(name="h1p", bufs=8) as h1_pool,
            tc.tile_pool(name="h2p", bufs=4) as h2_pool,
            tc.tile_pool(name="h3p", bufs=6) as h3_pool,
            tc.tile_pool(name="yo", bufs=3) as y_pool,
            tc.tile_pool(name="ps1", bufs=3, space="PSUM") as ps1,
            tc.tile_pool(name="ps2", bufs=1, space="PSUM") as ps2,
            tc.tile_pool(name="ps3", bufs=1, space="PSUM") as ps3,
            tc.tile_pool(name="ps4", bufs=1, space="PSUM") as ps4,
        ):
            warm = wpool.tile([H, TB], dt.bfloat16, tag="warm", name="warm")
            nc.gpsimd.memset(warm[:], 0)
            wps = ps1.tile([H, TB], dt.float32, tag="p1", name="p1_warm")
            for _ in range(7):
                nc.tensor.matmul(wps[:], warm[:, 0:H], warm[:], start=True, stop=True)

            # --- load L1 weights first on the fast HWDGE path (first-MM
            # gate), everything else on gpsimd SWDGE ---
            w1a_sb = wpool.tile([K1A, H], dt.bfloat16)
            nc.sync.dma_start(w1a_sb[:], w1t[0:K1A, :])
            w1b_sb = wpool.tile([K1B, H], dt.bfloat16)
            nc.sync.dma_start(w1b_sb[:], w1t[K1A : K1A + K1B, :])
            w2_sb = wpool.tile([H, H], dt.bfloat16)
            nc.gpsimd.dma_start(w2_sb[:], w2t[:, :])
            w3_sb = wpool.tile([H, H], dt.bfloat16)
            nc.gpsimd.dma_start(w3_sb[:], w3t[:, :])
            w4_sb = [
                wpool.tile([H, 32], dt.bfloat16, tag=f"w4_{j}", name=f"w4_{j}")
                for j in range(2)
            ]
            nc.gpsimd.dma_start(w4_sb[0][:], w4a[:, :])
            nc.gpsimd.dma_start(w4_sb[1][:], w4b[:, :])
            b_sb = []
            for j, bd in enumerate((b1d, b2d, b3d)):
                b = wpool.tile([H, 1], dt.float32, tag=f"b_{j}", name=f"b_{j}")
                nc.gpsimd.dma_start(b[:], bd[:, :])
                b_sb.append(b)

            def relu_evac(use_act, h_out, psum_in, bias_sb):
                if use_act:
                    return nc.scalar.activation(h_out[:], psum_in[:], relu, bias=bias_sb[:])
                else:
                    return nc.vector.tensor_scalar(
                        h_out[:],
                        psum_in[:],
                        bias_sb[:],
                        0.0,
                        mybir.AluOpType.add,
                        mybir.AluOpType.max,
                    )

            noload = []  # matmuls that reuse already-loaded weights
            xa_t: dict = {}
            xb_t: dict = {}
            h1_t: dict = {}
            h2_t: dict = {}
            h3_t: dict = {}

            def emit_load(hh, split=False):
                ch = slice(hh * HW, (hh + 1) * HW)
                xa = xa_pool.tile([K1A, HW], dt.bfloat16, tag="xa", name=f"xa_{hh}")
                xb = xb_pool.tile([K1B, HW], dt.bfloat16, tag="xb", name=f"xb_{hh}")
                if split:
                    # quarter-granularity writes so the first matmuls gate
                    # on a 256KB DMA, not the full half
                    qn = HW // 2
                    for qf in range(2):
                        cq = slice(hh * HW + qf * qn, hh * HW + (qf + 1) * qn)
                        nc.sync.dma_start(xa[:, qf * qn : (qf + 1) * qn], xTa[:, cq])
                        nc.sync.dma_start(xb[:, qf * qn : (qf + 1) * qn], xTb[:, cq])
                else:
                    nc.sync.dma_start(xa[:], xTa[:, ch])
                    nc.sync.dma_start(xb[:], xTb[:, ch])
                xa_t[hh], xb_t[hh] = xa, xb

            last_pe = [None]  # last PE instruction emitted this step

            def stage_A1(i):  # L1 part a for pair i: W1a(t0),W1a(t1)
                hh, sl = divmod(i, 2)
                xa = xa_t[hh]
                base = sl * (2 * TB)
                ps = []
                for q in range(2):
                    t = 2 * i + q
                    o = base + q * TB
                    p1 = ps1.tile([H, TB], dt.float32, tag="p1", name=f"p1_{t}")
                    mm = nc.tensor.matmul(
                        p1[:], w1a_sb[:], xa[:, o : o + TB], start=True, stop=False
                    )
                    if q == 1 and noload_opt:
                        mm.ins.ldweights = False
                        noload.append(mm.ins)
                    last_pe[0] = mm
                    ps.append((t, o, p1))
                return ps

            def stage_A2(i, ps):  # L1 part b: W1b(t0),W1b(t1)
                hh, sl = divmod(i, 2)
                xb = xb_t[hh]
                for qq, (t, o, p1) in enumerate(ps):
                    mm = nc.tensor.matmul(
                        p1[:], w1b_sb[:], xb[:, o : o + TB], start=False, stop=True
                    )
                    if qq == 1 and noload_opt:
                        mm.ins.ldweights = False
                        noload.append(mm.ins)
                    last_pe[0] = mm

            def evacs_A(i, ps):
                for t, o, p1 in ps:
                    h1 = h1_pool.tile([H, TB], dt.bfloat16, tag="h1", name=f"h1_{t}")
                    use_act = True if i % 8 == 7 else (t + i) % 2 == 0
                    relu_evac(use_act, h1, p1, b_sb[0])
                    h1_t[t] = h1

            def stage_B(i):  # L2 for pair i -> one 2-bank evac
                p2 = ps2.tile([H, 2 * TB], dt.float32, tag="p2", name=f"p2_{i}")
                for q in range(2):
                    t = 2 * i + q
                    h1 = h1_t.pop(t)
                    mm = nc.tensor.matmul(
                        p2[:, q * TB : (q + 1) * TB], w2_sb[:], h1[:],
                        start=True, stop=True,
                    )
                    if q == 1 and noload_opt:
                        mm.ins.ldweights = False
                        noload.append(mm.ins)
                    last_pe[0] = mm
                h2 = h2_pool.tile([H, 2 * TB], dt.bfloat16, tag="h2", name=f"h2_{i}")
                relu_evac(i % 2 == 0, h2, p2, b_sb[1])
                h2_t[i] = h2

            def stage_C(i):  # L3 for pair i -> one 2-bank evac
                p3 = ps3.tile([H, 2 * TB], dt.float32, tag="p3", name=f"p3_{i}")
                h2 = h2_t.pop(i)
                for q in range(2):
                    mm = nc.tensor.matmul(
                        p3[:, q * TB : (q + 1) * TB], w3_sb[:],
                        h2[:, q * TB : (q + 1) * TB],
                        start=True, stop=True,
                    )
                    if q == 1 and noload_opt:
                        mm.ins.ldweights = False
                        noload.append(mm.ins)
                    last_pe[0] = mm
                h3 = h3_pool.tile([H, 2 * TB], dt.bfloat16, tag="h3", name=f"h3_{i}")
                e3 = relu_evac(i % 2 == 1, h3, p3, b_sb[2])
                h3_t[i] = (h3, e3)

            p4_t: dict = {}

            def stage_Hj(pk, j):
                # head burst: variant j covers tiles 4j+u (u=0..3) of the
                # pack = pairs (4pk+2j, 4pk+2j+1), whose L3 evacs are >=2
                # steps old.  The burst is pinned contiguous behind the
                # step's last layer matmul with same-engine ordering deps,
                # so the scheduler cannot scatter the col-group LDWEIGHTS
                # between layer matmuls (each scatter serializes ~107ns).
                if j == 0:
                    p4_t[pk] = ps4.tile([H, TB], dt.float32, tag="p4", name=f"p4_{pk}")
                p4 = p4_t[pk]
                pairs = [4 * pk + 2 * j, 4 * pk + 2 * j + 1]
                hs = []
                e3s = []
                for pr in pairs:
                    h3, e3 = h3_t[pr]
                    hs.append(h3[:, 0:TB])
                    hs.append(h3[:, TB : 2 * TB])
                    e3s.append(e3)
                ldws = []
                for u in range(4):
                    ldw = nc.tensor.ldweights(
                        w4_sb[j][:], tile_position=(0, 32 * u)
                    )
                    for e3 in e3s:
                        _add_dep_helper(ldw.ins, e3.ins, True, "head ldw after e3")
                    if u == 0:
                        if last_pe[0] is not None:
                            _add_dep_helper(
                                ldw.ins, last_pe[0].ins, False, "pin head burst"
                            )
                    else:
                        _add_dep_helper(ldw.ins, ldws[-1].ins, False, "chain ldw")
                    ldws.append(ldw)
                mms = []
                for u in range(4):
                    mm = nc.tensor.matmul(
                        p4[32 * u : 32 * u + 32, :],
                        w4_sb[j][:],
                        hs[u],
                        start=(j == 0),
                        stop=(j == 1),
                        tile_position=(0, 32 * u),
                        skip_group_check=True,
                    )
                    mm.ins.ldweights = False
                    _add_dep_helper(mm.ins, ldws[u].ins, False, "head mm after ldw")
                    prev = mms[-1] if mms else ldws[-1]
                    _add_dep_helper(mm.ins, prev.ins, False, "chain head mm")
                    mms.append(mm)
                last_pe[0] = mms[-1]
                if j == 1:
                    for pr in [4 * pk, 4 * pk + 1] + pairs:
                        h3_t.pop(pr, None)

            def copy_store(pk):
                # one step after the pack's last head wave: the copy's
                # input is already complete, so it cannot block the Scalar
                # FIFO head and delay the critical evacuations behind it.
                p4 = p4_t.pop(pk)
                ysb = y_pool.tile([H, TB], dt.bfloat16, tag="ysb", name=f"ysb_{pk}")
                nc.scalar.copy(ysb[:], p4[:])
                # per-pack stores on GpSimd (SWDGE): small bursts that
                # never block load triggers on the Sync sequencer
                nc.gpsimd.dma_start(yTS[:, pk * TB : (pk + 1) * TB], ysb[:])

            # --- software-pipelined emission ---
            # PE stage order within a step is A, C, B (+ pinned head burst)
            # so the single-buffered ps2/ps3 evacuations get a full step of
            # slack before the next pair's matmuls need the banks back.
            # L1 evacuations (2 steps of slack) are emitted last so the
            # tight L2/L3 evacuations sit ahead of them in engine queues.
            # x halves: prime 6, then 1 every 2 steps, strictly in order.
            PRIME = 6
            emit_load(0, split=True)
            for hh in range(1, min(PRIME, n_halves)):
                emit_load(hh)
            for step in range(n_pairs + 7):
                if step % 2 == 0:
                    hh = PRIME + step // 2
                    if hh < n_halves:
                        emit_load(hh)
                if step < n_pairs:
                    ps_a = stage_A1(step)
                    stage_A2(step, ps_a)
                else:
                    ps_a = None
                ic = step - 4
                if 0 <= ic < n_pairs:
                    stage_C(ic)
                ib = step - 2
                if 0 <= ib < n_pairs:
                    stage_B(ib)
                iq = step - 6
                if 0 <= iq < n_pairs and iq % 2 == 1:
                    stage_Hj(iq // 4, (iq % 4) // 2)
                if ps_a is not None:
                    evacs_A(step, ps_a)
                if iq == n_pairs - 1:
                    copy_store(iq // 4)  # final pack: nothing left to block
                elif iq >= 4 and iq % 4 == 0 and iq // 4 - 1 < n_packs - 1:
                    copy_store(iq // 4 - 1)

    nc.compile()
    if noload_opt:
        try:
            _verify_noload_safety(nc, noload)
        except AssertionError:
            # schedule changed in a way that makes weight reuse unsafe;
            # rebuild without the optimization (correctness first)
            return build_nc(b_core, n_cores, noload_opt=False)
    return nc


def _weights_key(inst, idx):
    ap = inst.ins[idx]
    s = str(ap)
    return s


def _verify_noload_safety(nc, noload):
    """The schedule is static: verify no other weight load lands between a
    ldweights=False matmul and the instruction that loaded its weights."""
    import concourse.mybir as mybir

    noload_ids = {id(i) for i in noload}
    cur = None  # weights key currently in the PE array (full-array loads)
    checked = 0
    insts = []
    for blk in nc.m.functions[0].blocks:
        insts.extend(blk.instructions)
    for inst in insts:
        if inst.engine != mybir.EngineType.PE:
            continue
        kind = type(inst).__name__
        if kind == "InstLdweights":
            tp = getattr(inst, "tile_position", None)
            if not tp or tuple(tp) == (0, 0):
                cur = _weights_key(inst, 0)
            else:
                cur = ("coltile", None)  # partial col-group load
        elif kind == "InstMatmult":
            if id(inst) in noload_ids:
                want = _weights_key(inst, 1)
                assert cur == want, (
                    f"noload matmul {inst.name} expects weights {want}, array has {cur}"
                )
                checked += 1
            elif getattr(inst, "ldweights", None) is False:
                pass  # head matmul: guarded by its own explicit ldw deps
            else:
                tp = getattr(inst, "tile_position", None)
                if not tp or tuple(tp) == (0, 0):
                    cur = _weights_key(inst, 1)
                else:
                    cur = ("coltile", None)
    assert checked == len(noload), (checked, len(noload))


def _prep_core_inputs(x_shard: np.ndarray, weights: dict) -> dict:
    xT = x_shard.T.astype(BF16)
    xTa = np.ascontiguousarray(xT[0:K1A])
    xTb = np.zeros((K1B, x_shard.shape[0]), BF16)
    xTb[0 : D_IN - K1A] = xT[K1A:D_IN]
    return {"xTa": xTa, "xTb": xTb, **weights}


def _prep_weights(W1, b1, W2, b2, W3, b3, W4) -> dict:
    w4a = np.zeros((32, H), np.float32)
    w4a[0:D_OUT] = W4
    w4b = np.zeros((32, H), np.float32)
    w4b[D_OUT : 2 * D_OUT] = W4
    w1p = np.zeros((K1A + K1B, H), np.float32)
    w1p[0:D_IN] = np.sign(W1).T
    return {
        "w1t": w1p.astype(BF16),
        "w2t": np.ascontiguousarray(np.sign(W2).T).astype(BF16),
        "w3t": np.ascontiguousarray(np.sign(W3).T).astype(BF16),
        "w4a": np.ascontiguousarray(w4a.T).astype(BF16),
        "w4b": np.ascontiguousarray(w4b.T).astype(BF16),
        "b1": b1.reshape(H, 1).astype(np.float32),
        "b2": b2.reshape(H, 1).astype(np.float32),
        "b3": b3.reshape(H, 1).astype(np.float32),
    }


def _unscramble(yTS: np.ndarray, b_core: int) -> np.ndarray:
    """yTS [128, n_packs*TB] strip layout -> y_core [b_core, 10] (fp32)."""
    n_packs = b_core // (PACK * TB)
    # yTS[32u+10j+p, pk*TB+c] = y[(pk*8+4j+u)*TB + c, p]
    v = yTS.astype(np.float32).reshape(4, 32, n_packs, TB)[:, :20]  # [u, 10j+p, pk, c]
    v = v.reshape(4, 2, 10, n_packs, TB)  # [u, j, p, pk, c]
    # -> y[pk, j, u, c, p]
    y = v.transpose(3, 1, 0, 4, 2).reshape(b_core, D_OUT)
    return y


_NC_CACHE: dict = {}


def run(x, W1, b1, W2, b2, W3, b3, W4, b4, trace=False, trace_kwargs=None):
    """Run the SPMD kernel on 8 cores; returns (y, BassKernelResults)."""
    x = np.asarray(x, dtype=np.float32)
    b_total = x.shape[0]
    assert b_total % N_CORES == 0
    b_core = b_total // N_CORES

    key = b_core
    if key not in _NC_CACHE:
        _NC_CACHE[key] = build_nc(b_core)
    nc = _NC_CACHE[key]

    weights = _prep_weights(
        np.asarray(W1), np.asarray(b1), np.asarray(W2), np.asarray(b2),
        np.asarray(W3), np.asarray(b3), np.asarray(W4),
    )
    in_maps = [
        _prep_core_inputs(x[c * b_core : (c + 1) * b_core], weights)
        for c in range(N_CORES)
    ]
    res = run_bass_kernel_spmd(
        nc,
        in_maps,
        list(range(N_CORES)),
        trace=trace,
        **(trace_kwargs or {}),
    )
    b4f = np.asarray(b4, dtype=np.float32)
    y = np.empty((b_total, D_OUT), dtype=np.float32)
    for c in range(N_CORES):
        y[c * b_core : (c + 1) * b_core] = _unscramble(res.results[c]["yTS"], b_core)
    y += b4f
    return y, res


def kernel(x, W1, b1, W2, b2, W3, b3, W4, b4):
    y, _ = run(x, W1, b1, W2, b2, W3, b3, W4, b4)
    return y
